# revision 10
# baseline (speedup 1.0000x reference)
"""Distance-loss kernel for Trainium2 (8 NeuronCores, data-parallel over batch).

loss = mean over (b, c != label_b) of sqrt(||x_b - center_c||^2)

Host-side staging is layout/dtype only (fp8/bf16 casts + transposed
tiled views); all arithmetic (norms, matmuls, sqrt, reductions,
correction) runs on device with fp32 accumulation. x-hat = e4m3(x) and
c-hat = e4m3(centers) are used CONSISTENTLY: the matmul operands ship
as fp8, every other staged view (row tiles for norms/correction,
gather source) ships as bf16 of the same fp8-rounded values, so
d^2 = ||x||^2 + ||c||^2 - 2 x.c is exact in expectation and the only
error is the (tiny, averaged-out) quantization of the points.

Per-core plan (B_shard = 2048 rows, distmat computed as psum[c, b]):
  - psum[c, b] = c_c . x_b + (-1/2)||x_b||^2 via PE matmuls (fp8
    operands, fp32 psum). Centers ship RAW: the ScalarE activation
    computes d = sqrt(-2 * psum + ||c_c||^2) with scale=-2 and the
    class norm as per-partition bias, accumulating sum_b d.
  - ||x||^2 without transpose or DRAM round-trip: square the resident
    x^T tiles on VectorE, ones-column matmuls reduce over d into one
    psum row [1, 2048], one ScalarE copy with scale=-0.5 makes the
    bf16 aug row that rides each c-tile's psum as K=1 matmuls.
  - label-entry correction: indirect-DMA gather of centers[labels]
    (labels land via the faster-spinning sync queue), sum_d (x-g)^2 on
    VectorE, one sqrt at the end whose output aliases the last d-tile
    (WAW) so the scheduler cannot hoist it ahead of the main sqrts.
  - A PE warm-up burst runs while DMAs stream so the clock governor
    reaches full speed before the real matmuls.
  - host sums the 8 per-core partials and divides by B*(C-1).
"""

import sys
from contextlib import ExitStack

import numpy as np

if "/opt/trn_rl_repo" not in sys.path:
    sys.path.insert(0, "/opt/trn_rl_repo")

import ml_dtypes

import concourse.bass as bass
import concourse.mybir as mybir
from concourse.bacc import Bacc
from concourse.bass import IndirectOffsetOnAxis
from concourse.tile import TileContext

F32 = mybir.dt.float32
BF16 = mybir.dt.bfloat16
FP8 = mybir.dt.float8e4
I32 = mybir.dt.int32
AF = mybir.ActivationFunctionType
ALU = mybir.AluOpType
BF = ml_dtypes.bfloat16
F8 = ml_dtypes.float8_e4m3

N_CORES = 8
B = 16384
C = 1000
D = 256
BS = B // N_CORES          # 2048 rows per core
T = BS // 128              # 16 b-tiles per core
NC_TILES = 8               # ceil(C / 128) class tiles


def build_nc() -> bass.Bass:
    nc = Bacc()
    # xT0/xT1: x^T halves (fp8)  xTi[p, b] = x[b, i*128+p]
    # cT : centers^T (fp8, raw)  cT[p, i*C+c] = centers[c, i*128+p]
    # cp : center rows (bf16)    cp[p, i*D:(i+1)*D] = centers[i*128+p, :]
    # xp0/xp1: x rows (bf16)     xpH[p, t*D:(t+1)*D] = x[(8H+t)*128+p, :]
    # cb : centers rows (bf16), indirect-gather source
    xT0_d = nc.dram_tensor("xT0", [128, BS], FP8, kind="ExternalInput")
    xT1_d = nc.dram_tensor("xT1", [128, BS], FP8, kind="ExternalInput")
    cT_d = nc.dram_tensor("cT", [128, 2 * C], FP8, kind="ExternalInput")
    cp_d = nc.dram_tensor("cp", [128, NC_TILES * D], BF16, kind="ExternalInput")
    xp0_d = nc.dram_tensor("xp0", [128, 8 * D], BF16, kind="ExternalInput")
    xp1_d = nc.dram_tensor("xp1", [128, 8 * D], BF16, kind="ExternalInput")
    cb_d = nc.dram_tensor("cb", [C, D], BF16, kind="ExternalInput")
    l_d = nc.dram_tensor("labels", [128, T], I32, kind="ExternalInput")
    o_d = nc.dram_tensor("out", [1, 1], F32, kind="ExternalOutput")

    with TileContext(nc) as tc, ExitStack() as ctx:
        const = ctx.enter_context(tc.tile_pool(name="const", bufs=1))
        sqpool = ctx.enter_context(tc.tile_pool(name="sqpool", bufs=2))
        cqpool = ctx.enter_context(tc.tile_pool(name="cqpool", bufs=2))
        xpool = ctx.enter_context(tc.tile_pool(name="xpool", bufs=3))
        dpool = ctx.enter_context(tc.tile_pool(name="dpool", bufs=2))
        mmps = ctx.enter_context(tc.tile_pool(name="mmps", bufs=2, space="PSUM"))

        # labels ride the sync queue (it spins up ~1.7us earlier than the
        # scalar queue) so the 16-gather chain on GpSimd starts ASAP.
        lab_sb = const.tile([128, T], I32)
        nc.sync.dma_start(out=lab_sb[:], in_=l_d[:, :])
        g_sb = const.tile([128, T * D], BF16)
        for t in range(T):
            nc.gpsimd.indirect_dma_start(
                out=g_sb[:, t * D : (t + 1) * D],
                out_offset=None,
                in_=cb_d[:, :],
                in_offset=IndirectOffsetOnAxis(ap=lab_sb[:, t : t + 1], axis=0),
            )

        # input DMAs, balanced across the two HWDGE queues
        xT0 = const.tile([128, BS], FP8)
        nc.sync.dma_start(out=xT0[:], in_=xT0_d[:, :])
        xT1 = const.tile([128, BS], FP8)
        nc.scalar.dma_start(out=xT1[:], in_=xT1_d[:, :])
        cTf = const.tile([128, 2 * C], FP8)
        nc.scalar.dma_start(out=cTf[:], in_=cT_d[:, :])
        xp1 = const.tile([128, 8 * D], BF16)
        nc.sync.dma_start(out=xp1[:], in_=xp1_d[:, :])
        cperm = const.tile([128, NC_TILES * D], BF16)
        nc.scalar.dma_start(out=cperm[:], in_=cp_d[:, :])
        xp0 = const.tile([128, 8 * D], BF16)
        nc.scalar.dma_start(out=xp0[:], in_=xp0_d[:, :])
        xTs = [xT0, xT1]

        def x_slice(t):
            return (xp0 if t < 8 else xp1)[:, (t % 8) * D : (t % 8 + 1) * D]

        # constants on VectorE (queue head, no deps)
        dum0 = const.tile([128, 1], F32)
        nc.vector.memset(dum0[:], 1.0)
        wu_w = const.tile([128, 4], BF16)
        nc.vector.memset(wu_w[:], 0.5)
        wu_r = const.tile([128, 512], BF16)
        nc.vector.memset(wu_r[:], 0.25)
        ones1 = const.tile([1, 128], BF16)
        nc.vector.memset(ones1[:], 1.0)
        onesk = const.tile([128, 1], BF16)
        nc.vector.memset(onesk[:], 1.0)
        acc = const.tile([128, NC_TILES], F32)       # sum_b sqrt(dist)
        nc.vector.memset(acc[:], 0.0)

        # dummy sqrt: forces both ACT tables (Sqrt via dummy, Copy via
        # the xx cast) resident before the main loop needs them.
        dum1 = const.tile([128, 1], F32)
        nc.scalar.activation(dum1[:], dum0[:], AF.Sqrt)

        # PE warm-up burst while DMAs stream; writes junk into the psum
        # row the ones-matmuls later reset via start=True.
        psxx = mmps.tile([128, 2048], F32, tag="mm")
        for rep in range(8):
            nc.tensor.matmul(psxx[0:4, 0:512], wu_w[:], wu_r[:],
                             start=(rep == 0), stop=(rep == 7))

        # ||x||^2 in matmul orientation: square x^T halves on VectorE,
        # reduce over d via ones-column matmuls into psxx[0, j*512:...].
        sq = []
        for i in range(2):
            sq_t = sqpool.tile([128, BS], BF16, tag="sq")
            nc.vector.tensor_tensor(out=sq_t[:], in0=xTs[i][:], in1=xTs[i][:],
                                    op=ALU.mult)
            sq.append(sq_t)
        for i in range(2):
            for j in range(4):
                nc.tensor.matmul(
                    psxx[0:1, j * 512 : (j + 1) * 512],
                    onesk[:],
                    sq[i][:, j * 512 : (j + 1) * 512],
                    start=(i == 0), stop=(i == 1),
                )

        # aug row: xxrow[0, b] = -0.5 * ||x_b||^2 (bf16)
        xxrow = const.tile([1, BS], BF16)
        nc.scalar.activation(xxrow[:], psxx[0:1, :], AF.Copy, scale=-0.5)

        # ||c||^2 per class tile (fp32, separate tiles so ACT m only
        # waits on its own column, not all eight)
        ccs = []
        for i in range(NC_TILES):
            cc_i = const.tile([128, 1], F32, tag=f"cc_{i}")
            csq = cqpool.tile([128, D], BF16, tag="csq")
            nc.vector.scalar_tensor_tensor(
                out=csq[:], in0=cperm[:, i * D : (i + 1) * D], scalar=0.0,
                in1=cperm[:, i * D : (i + 1) * D],
                op0=ALU.bypass, op1=ALU.mult,
                accum_out=cc_i[:],
            )
            ccs.append(cc_i)

        # main loop: per c-tile, 8 k-matmuls + 4 aug matmuls, then
        # ScalarE sqrt(-2*psum + ||c||^2) with row-sum accumulation.
        dt_last = None
        for m in range(NC_TILES):
            cnt = min(128, C - m * 128)
            ps = mmps.tile([128, 2048], F32, tag="mm")
            for i in range(2):
                for j in range(4):
                    nc.tensor.matmul(
                        ps[0:cnt, j * 512 : (j + 1) * 512],
                        cTf[:, i * C + m * 128 : i * C + m * 128 + cnt],
                        xTs[i][:, j * 512 : (j + 1) * 512],
                        start=(i == 0), stop=False,
                    )
            for j in range(4):
                nc.tensor.matmul(
                    ps[0:cnt, j * 512 : (j + 1) * 512],
                    ones1[:, 0:cnt],
                    xxrow[:, j * 512 : (j + 1) * 512],
                    start=False, stop=(j == 3),
                )
            dt_ = dpool.tile([128, 2048], F32, tag="d")
            nc.scalar.activation(
                dt_[0:cnt, :], ps[0:cnt, :], AF.Sqrt,
                bias=ccs[m][0:cnt, :], scale=-2.0,
                accum_out=acc[0:cnt, m : m + 1],
            )
            dt_last = dt_

        # label-entry correction (bf16 operands, fp32 accumulation)
        dacc = const.tile([128, T], F32)             # label-entry dist^2
        for t in range(T):
            df = xpool.tile([128, D], BF16, tag="df")
            nc.vector.tensor_sub(df[:], x_slice(t),
                                 g_sb[:, t * D : (t + 1) * D])
            dfsq = xpool.tile([128, D], BF16, tag="dfsq")
            nc.vector.scalar_tensor_tensor(
                out=dfsq[:], in0=df[:], scalar=0.0, in1=df[:],
                op0=ALU.bypass, op1=ALU.mult, accum_out=dacc[:, t : t + 1],
            )

        # corr sqrt writes into the tail of the last d-tile: the WAW dep
        # pins it AFTER the m=7 sqrt in the ACT queue, so a slow gather
        # chain can never block the main-loop sqrts behind it.
        corp = const.tile([128, 1], F32)
        nc.scalar.activation(dt_last[:, 2048 - T :], dacc[:], AF.Sqrt,
                             accum_out=corp[:])
        totp = const.tile([128, 1], F32)
        nc.vector.reduce_sum(out=totp[:], in_=acc[:], axis=mybir.AxisListType.X)
        part = const.tile([128, 1], F32)
        nc.vector.tensor_sub(part[:], totp[:], corp[:])
        ones_col = const.tile([128, 1], F32)
        nc.vector.memset(ones_col[:], 1.0)
        red_ps = mmps.tile([128, 2048], F32, tag="mm")
        nc.tensor.matmul(red_ps[0:1, 0:1], ones_col[:], part[:],
                         start=True, stop=True)
        red = const.tile([1, 1], F32)
        nc.scalar.copy(red[:], red_ps[0:1, 0:1])
        nc.sync.dma_start(out=o_d[0:1, 0:1], in_=red[0:1, 0:1])

    nc.compile()
    return nc


_NC_CACHE = None


def _get_nc():
    global _NC_CACHE
    if _NC_CACHE is None:
        _NC_CACHE = build_nc()
    return _NC_CACHE


def make_in_maps(x, centers, labels):
    x = np.asarray(x, dtype=np.float32)
    centers = np.asarray(centers, dtype=np.float32)
    labels = np.asarray(labels)
    # fp8-rounded points, used consistently by every staged view
    c8 = centers.astype(F8)
    cb = c8.astype(BF)
    cT = np.ascontiguousarray(
        c8.T.reshape(2, 128, C).transpose(1, 0, 2).reshape(128, 2 * C)
    )
    cpad = np.zeros((NC_TILES * 128, D), F8)
    cpad[:C] = c8
    cp = np.ascontiguousarray(
        cpad.reshape(NC_TILES, 128, D).transpose(1, 0, 2).reshape(128, -1)
    ).astype(BF)
    in_maps = []
    for i in range(N_CORES):
        x8 = x[i * BS : (i + 1) * BS].astype(F8)
        xT = np.ascontiguousarray(x8.T)  # [D, BS] fp8
        xp = np.ascontiguousarray(
            x8.reshape(T, 128, D).transpose(1, 0, 2).reshape(128, -1)
        ).astype(BF)
        ls = labels[i * BS : (i + 1) * BS].astype(np.int32)
        # lab[p, t] = label of shard row t*128 + p (indirect-gather order)
        lab = np.ascontiguousarray(ls.reshape(T, 128).T)
        in_maps.append({
            "xT0": xT[0:128], "xT1": xT[128:256], "cT": cT, "cp": cp,
            "xp0": xp[:, : 8 * D], "xp1": xp[:, 8 * D :],
            "cb": cb, "labels": lab,
        })
    return in_maps


def _ensure_ntff_hook_module():
    """Provide antenv.axon_hooks if the image's antenv package lacks it.

    concourse.bass_utils imports it for trace=True under axon; the hook
    itself lives in libaxon_pjrt.so and is wrapped by trn_agent_boot.
    """
    import types

    try:
        import antenv.axon_hooks  # noqa: F401
        return
    except ImportError:
        pass
    mod = types.ModuleType("antenv.axon_hooks")
    state = {"hook": None}

    def set_axon_ntff_profile_hook(hook):
        state["hook"] = hook

    def get_axon_ntff_profile_hook():
        if state["hook"] is None:
            try:
                from trn_agent_boot.trn_boot import _ntff_profile_via_ctypes

                state["hook"] = _ntff_profile_via_ctypes(
                    "/opt/axon/libaxon_pjrt.so"
                )
            except Exception:
                return None
        return state["hook"]

    mod.set_axon_ntff_profile_hook = set_axon_ntff_profile_hook
    mod.get_axon_ntff_profile_hook = get_axon_ntff_profile_hook
    sys.modules["antenv.axon_hooks"] = mod
    try:
        import antenv

        antenv.axon_hooks = mod
    except ImportError:
        pass


def kernel(x, centers, labels, _results_out=None, **run_kwargs):
    _ensure_ntff_hook_module()
    from concourse.bass_utils import run_bass_kernel_spmd

    nc = _get_nc()
    in_maps = make_in_maps(x, centers, labels)
    res = run_bass_kernel_spmd(nc, in_maps, core_ids=list(range(N_CORES)),
                               **run_kwargs)
    if _results_out is not None:
        _results_out.append(res)
    partials = [float(r["out"][0, 0]) for r in res.results]
    total = float(np.sum(np.asarray(partials, dtype=np.float64)))
    loss = total / (B * (C - 1))
    return np.float32(loss)


# revision 13
# speedup vs baseline: 1.1525x; 1.1525x over previous
"""Distance-loss kernel for Trainium2 (8 NeuronCores, data-parallel over batch).

loss = mean over (b, c != label_b) of sqrt(||x_b - center_c||^2)

Host-side staging is layout/dtype only (bf16 casts + transposed tiled
views); all arithmetic (norms, matmuls, sqrt, reductions, correction)
runs on device with fp32 accumulation.

Per-core plan (B_shard = 2048 rows, distmat computed as psum[c, b]):
  - psum[c, b] = c_c . x_b + (-1/2)||x_b||^2 via PE matmuls in bf16.
    Centers ship RAW: the ScalarE activation computes
    d = sqrt(-2 * psum + ||c_c||^2) with scale=-2 and the class norm as
    per-partition bias, accumulating sum_b d.
  - ||x||^2 without transpose or DRAM round-trip: square the resident
    x^T quarter-tiles on VectorE as they land, ones-column matmuls
    reduce over d into one psum row [1, 2048], four ScalarE copies with
    scale=-0.5 make the bf16 aug row slices that ride each c-tile's
    psum as K=1 matmuls.
  - label-entry correction: indirect-DMA gather of centers[labels]
    (labels land via the faster-spinning sync queue), sum_d (x-g)^2 on
    VectorE, one sqrt at the end whose output lives in the same tile as
    the main accumulators (WAW) so the scheduler cannot hoist it ahead
    of the main-loop sqrts when the gather chain runs long.
  - A PE warm-up burst runs while DMAs stream so the clock governor
    reaches full speed; its tail is four fp8 DoubleRow matmuls on junk
    operands, timing probes for a possible fp8 main loop.
  - host sums the 8 per-core partials and divides by B*(C-1).
"""

import sys
from contextlib import ExitStack

import numpy as np

if "/opt/trn_rl_repo" not in sys.path:
    sys.path.insert(0, "/opt/trn_rl_repo")

import ml_dtypes

import concourse.bass as bass
import concourse.mybir as mybir
from concourse.bacc import Bacc
from concourse.bass import IndirectOffsetOnAxis
from concourse.tile import TileContext

F32 = mybir.dt.float32
BF16 = mybir.dt.bfloat16
FP8 = mybir.dt.float8e4
I32 = mybir.dt.int32
AF = mybir.ActivationFunctionType
ALU = mybir.AluOpType
PM = mybir.MatmulPerfMode
BF = ml_dtypes.bfloat16

N_CORES = 8
B = 16384
C = 1000
D = 256
BS = B // N_CORES          # 2048 rows per core
T = BS // 128              # 16 b-tiles per core
NC_TILES = 8               # ceil(C / 128) class tiles
HB = BS // 2               # 1024-column b-half


def build_nc() -> bass.Bass:
    nc = Bacc()
    # xTia/xTib: x^T quarters   xTiH[p, b] = x[H*1024+b, i*128+p]
    # cT : centers^T (raw)      cT[p, i*C+c] = centers[c, i*128+p]
    # cpA/cpB: center rows      cpH[p, i*D:(i+1)*D] = centers[(4H+i)*128+p, :]
    # xp0/xp1: x rows, sbuf     xpH[p, t*D:(t+1)*D] = x[(8H+t)*128+p, :]
    # cb : centers rows (indirect-gather source)
    xt_d = {}
    for i in range(2):
        for h, hn in enumerate("ab"):
            xt_d[i, h] = nc.dram_tensor(f"xT{i}{hn}", [128, HB], BF16,
                                        kind="ExternalInput")
    cT_d = nc.dram_tensor("cT", [128, 2 * C], BF16, kind="ExternalInput")
    cpA_d = nc.dram_tensor("cpA", [128, 4 * D], BF16, kind="ExternalInput")
    cpB_d = nc.dram_tensor("cpB", [128, 4 * D], BF16, kind="ExternalInput")
    xp0_d = nc.dram_tensor("xp0", [128, 8 * D], BF16, kind="ExternalInput")
    xp1_d = nc.dram_tensor("xp1", [128, 8 * D], BF16, kind="ExternalInput")
    cb_d = nc.dram_tensor("cb", [C, D], BF16, kind="ExternalInput")
    l_d = nc.dram_tensor("labels", [128, T], I32, kind="ExternalInput")
    o_d = nc.dram_tensor("out", [1, 1], F32, kind="ExternalOutput")

    with TileContext(nc) as tc, ExitStack() as ctx:
        const = ctx.enter_context(tc.tile_pool(name="const", bufs=1))
        cqpool = ctx.enter_context(tc.tile_pool(name="cqpool", bufs=2))
        xpool = ctx.enter_context(tc.tile_pool(name="xpool", bufs=3))
        dpool = ctx.enter_context(tc.tile_pool(name="dpool", bufs=2))
        mmps = ctx.enter_context(tc.tile_pool(name="mmps", bufs=2, space="PSUM"))

        # labels ride the sync queue (it spins up ~1.7us earlier than the
        # scalar queue) so the 16-gather chain on GpSimd starts ASAP.
        lab_sb = const.tile([128, T], I32)
        nc.sync.dma_start(out=lab_sb[:], in_=l_d[:, :])
        g_sb = const.tile([128, T * D], BF16)
        for t in range(T):
            nc.gpsimd.indirect_dma_start(
                out=g_sb[:, t * D : (t + 1) * D],
                out_offset=None,
                in_=cb_d[:, :],
                in_offset=IndirectOffsetOnAxis(ap=lab_sb[:, t : t + 1], axis=0),
            )

        # input DMAs: a-halves (j=0,1 columns) of both k-groups early on
        # the sync queue, cT first on scalar; b-halves follow.
        xts = {}
        for key, src, q in (
            ((0, 0), xt_d[0, 0], nc.sync),
            ((1, 0), xt_d[1, 0], nc.sync),
            ((0, 1), xt_d[0, 1], nc.scalar),
            ((1, 1), xt_d[1, 1], nc.scalar),
        ):
            t_ = const.tile([128, HB], BF16, tag=f"xT{key[0]}{key[1]}")
            q.dma_start(out=t_[:], in_=src[:, :])
            xts[key] = t_
        cTf = const.tile([128, 2 * C], BF16)
        nc.scalar.dma_start(out=cTf[:], in_=cT_d[:, :])
        cpA = const.tile([128, 4 * D], BF16)
        nc.sync.dma_start(out=cpA[:], in_=cpA_d[:, :])
        cpB = const.tile([128, 4 * D], BF16)
        nc.scalar.dma_start(out=cpB[:], in_=cpB_d[:, :])
        xp1 = const.tile([128, 8 * D], BF16)
        nc.sync.dma_start(out=xp1[:], in_=xp1_d[:, :])
        xp0 = const.tile([128, 8 * D], BF16)
        nc.scalar.dma_start(out=xp0[:], in_=xp0_d[:, :])

        def xt_slice(i, j):
            # [128, 512] slice of the x^T quarter holding columns j*512..
            return xts[i, j // 2][:, (j % 2) * 512 : (j % 2 + 1) * 512]

        def x_slice(t):
            return (xp0 if t < 8 else xp1)[:, (t % 8) * D : (t % 8 + 1) * D]

        # constants on VectorE (queue head, no deps)
        dum0 = const.tile([128, 1], F32)
        nc.vector.memset(dum0[:], 1.0)
        wu_w = const.tile([128, 4], BF16)
        nc.vector.memset(wu_w[:], 0.5)
        wu_r = const.tile([128, 512], BF16)
        nc.vector.memset(wu_r[:], 0.25)
        wu8w = const.tile([128, 2, 128], FP8)
        nc.vector.memset(wu8w[:], 0.5)
        wu8r = const.tile([128, 2, 512], FP8)
        nc.vector.memset(wu8r[:], 0.25)
        ones1 = const.tile([1, 128], BF16)
        nc.vector.memset(ones1[:], 1.0)
        onesk = const.tile([128, 1], BF16)
        nc.vector.memset(onesk[:], 1.0)
        # acc2 cols 0:8 = per-c-tile sum_b sqrt(dist); cols 8:24 = the
        # correction sqrt outputs (shared tile => WAW pins corr last)
        acc2 = const.tile([128, NC_TILES + T], F32)
        nc.vector.memset(acc2[:], 0.0)

        # dummy sqrt: loads the Sqrt ACT table before the loop needs it
        dum1 = const.tile([128, 1], F32)
        nc.scalar.activation(dum1[:], dum0[:], AF.Sqrt)

        # PE warm-up burst while DMAs stream; writes junk into the psum
        # row the ones-matmuls later reset via start=True. The last four
        # reps are fp8 DoubleRow probes (junk data, timing only).
        psxx = mmps.tile([128, 2048], F32, tag="mm")
        for rep in range(8):
            nc.tensor.matmul(psxx[0:4, 0:512], wu_w[:], wu_r[:],
                             start=(rep == 0), stop=False)
        for rep in range(4):
            nc.tensor.matmul(psxx[0:128, 0:512], wu8w[:], wu8r[:],
                             start=False, stop=(rep == 3),
                             perf_mode=PM.DoubleRow)

        # ||x||^2 in matmul orientation: square x^T quarters on VectorE
        # as they land, ones-column matmuls reduce over d into
        # psxx[0, j*512:...], one bf16 cast per j with scale=-0.5.
        sqs = {}
        for i, h in ((0, 0), (1, 0), (0, 1), (1, 1)):
            sq_t = cqpool.tile([128, HB], BF16, tag=f"sq{i}{h}")
            nc.vector.tensor_tensor(out=sq_t[:], in0=xts[i, h][:],
                                    in1=xts[i, h][:], op=ALU.mult)
            sqs[i, h] = sq_t
        xx4 = []
        for j in range(4):
            for i in range(2):
                nc.tensor.matmul(
                    psxx[0:1, j * 512 : (j + 1) * 512],
                    onesk[:],
                    sqs[i, j // 2][:, (j % 2) * 512 : (j % 2 + 1) * 512],
                    start=(i == 0), stop=(i == 1),
                )
            xxj = const.tile([1, 512], BF16, tag=f"xx4_{j}")
            nc.scalar.activation(xxj[:], psxx[0:1, j * 512 : (j + 1) * 512],
                                 AF.Copy, scale=-0.5)
            xx4.append(xxj)

        # ||c||^2 per class tile (separate tiles so ACT m only waits on
        # its own column); cpA covers tiles 0-3, cpB tiles 4-7.
        ccs = []
        for i in range(NC_TILES):
            src = (cpA if i < 4 else cpB)[:, (i % 4) * D : (i % 4 + 1) * D]
            cc_i = const.tile([128, 1], F32, tag=f"cc_{i}")
            csq = cqpool.tile([128, D], BF16, tag="csq")
            nc.vector.scalar_tensor_tensor(
                out=csq[:], in0=src, scalar=0.0, in1=src,
                op0=ALU.bypass, op1=ALU.mult,
                accum_out=cc_i[:],
            )
            ccs.append(cc_i)

        # main loop: per c-tile, 8 k-matmuls + 4 aug matmuls, then
        # ScalarE sqrt(-2*psum + ||c||^2) with row-sum accumulation.
        for m in range(NC_TILES):
            cnt = min(128, C - m * 128)
            ps = mmps.tile([128, 2048], F32, tag="mm")
            for j in range(4):
                for i in range(2):
                    nc.tensor.matmul(
                        ps[0:cnt, j * 512 : (j + 1) * 512],
                        cTf[:, i * C + m * 128 : i * C + m * 128 + cnt],
                        xt_slice(i, j),
                        start=(i == 0), stop=False,
                    )
            for j in range(4):
                nc.tensor.matmul(
                    ps[0:cnt, j * 512 : (j + 1) * 512],
                    ones1[:, 0:cnt],
                    xx4[j][:],
                    start=False, stop=(j == 3),
                )
            dt_ = dpool.tile([128, 2048], BF16, tag="d")
            nc.scalar.activation(
                dt_[0:cnt, :], ps[0:cnt, :], AF.Sqrt,
                bias=ccs[m][0:cnt, :], scale=-2.0,
                accum_out=acc2[0:cnt, m : m + 1],
            )

        # label-entry correction (bf16 operands, fp32 accumulation)
        dacc = const.tile([128, T], F32)             # label-entry dist^2
        for t in range(T):
            df = xpool.tile([128, D], BF16, tag="df")
            nc.vector.tensor_sub(df[:], x_slice(t),
                                 g_sb[:, t * D : (t + 1) * D])
            dfsq = xpool.tile([128, D], BF16, tag="dfsq")
            nc.vector.scalar_tensor_tensor(
                out=dfsq[:], in0=df[:], scalar=0.0, in1=df[:],
                op0=ALU.bypass, op1=ALU.mult, accum_out=dacc[:, t : t + 1],
            )

        # corr sqrt: output shares acc2 (WAW with the m=7 accumulate) so
        # it schedules after the main-loop sqrts; accum_out = sum_t.
        corp = const.tile([128, 1], F32)
        nc.scalar.activation(acc2[:, NC_TILES:], dacc[:], AF.Sqrt,
                             accum_out=corp[:])
        totp = const.tile([128, 1], F32)
        nc.vector.reduce_sum(out=totp[:], in_=acc2[:, 0:NC_TILES],
                             axis=mybir.AxisListType.X)
        part = const.tile([128, 1], F32)
        nc.vector.tensor_sub(part[:], totp[:], corp[:])
        ones_col = const.tile([128, 1], F32)
        nc.vector.memset(ones_col[:], 1.0)
        red_ps = mmps.tile([128, 2048], F32, tag="mm")
        nc.tensor.matmul(red_ps[0:1, 0:1], ones_col[:], part[:],
                         start=True, stop=True)
        red = const.tile([1, 1], F32)
        nc.scalar.copy(red[:], red_ps[0:1, 0:1])
        nc.sync.dma_start(out=o_d[0:1, 0:1], in_=red[0:1, 0:1])

    nc.compile()
    return nc


_NC_CACHE = None


def _get_nc():
    global _NC_CACHE
    if _NC_CACHE is None:
        _NC_CACHE = build_nc()
    return _NC_CACHE


def make_in_maps(x, centers, labels):
    x = np.asarray(x, dtype=np.float32)
    centers = np.asarray(centers, dtype=np.float32)
    labels = np.asarray(labels)
    cb = centers.astype(BF)
    cT = np.ascontiguousarray(
        centers.T.reshape(2, 128, C).transpose(1, 0, 2).reshape(128, 2 * C)
    ).astype(BF)
    cpad = np.zeros((NC_TILES * 128, D), np.float32)
    cpad[:C] = centers
    cp = np.ascontiguousarray(
        cpad.reshape(NC_TILES, 128, D).transpose(1, 0, 2).reshape(128, -1)
    ).astype(BF)
    in_maps = []
    for i in range(N_CORES):
        xs = x[i * BS : (i + 1) * BS]
        xT = np.ascontiguousarray(xs.T).astype(BF)  # [D, BS]
        xp = np.ascontiguousarray(
            xs.reshape(T, 128, D).transpose(1, 0, 2).reshape(128, -1)
        ).astype(BF)
        ls = labels[i * BS : (i + 1) * BS].astype(np.int32)
        # lab[p, t] = label of shard row t*128 + p (indirect-gather order)
        lab = np.ascontiguousarray(ls.reshape(T, 128).T)
        in_maps.append({
            "xT0a": xT[0:128, 0:HB], "xT0b": xT[0:128, HB:],
            "xT1a": xT[128:256, 0:HB], "xT1b": xT[128:256, HB:],
            "cT": cT, "cpA": cp[:, : 4 * D], "cpB": cp[:, 4 * D :],
            "xp0": xp[:, : 8 * D], "xp1": xp[:, 8 * D :],
            "cb": cb, "labels": lab,
        })
    return in_maps


def _ensure_ntff_hook_module():
    """Provide antenv.axon_hooks if the image's antenv package lacks it.

    concourse.bass_utils imports it for trace=True under axon; the hook
    itself lives in libaxon_pjrt.so and is wrapped by trn_agent_boot.
    """
    import types

    try:
        import antenv.axon_hooks  # noqa: F401
        return
    except ImportError:
        pass
    mod = types.ModuleType("antenv.axon_hooks")
    state = {"hook": None}

    def set_axon_ntff_profile_hook(hook):
        state["hook"] = hook

    def get_axon_ntff_profile_hook():
        if state["hook"] is None:
            try:
                from trn_agent_boot.trn_boot import _ntff_profile_via_ctypes

                state["hook"] = _ntff_profile_via_ctypes(
                    "/opt/axon/libaxon_pjrt.so"
                )
            except Exception:
                return None
        return state["hook"]

    mod.set_axon_ntff_profile_hook = set_axon_ntff_profile_hook
    mod.get_axon_ntff_profile_hook = get_axon_ntff_profile_hook
    sys.modules["antenv.axon_hooks"] = mod
    try:
        import antenv

        antenv.axon_hooks = mod
    except ImportError:
        pass


def kernel(x, centers, labels, _results_out=None, **run_kwargs):
    _ensure_ntff_hook_module()
    from concourse.bass_utils import run_bass_kernel_spmd

    nc = _get_nc()
    in_maps = make_in_maps(x, centers, labels)
    res = run_bass_kernel_spmd(nc, in_maps, core_ids=list(range(N_CORES)),
                               **run_kwargs)
    if _results_out is not None:
        _results_out.append(res)
    partials = [float(r["out"][0, 0]) for r in res.results]
    total = float(np.sum(np.asarray(partials, dtype=np.float64)))
    loss = total / (B * (C - 1))
    return np.float32(loss)


# revision 14
# speedup vs baseline: 1.3311x; 1.1550x over previous
"""Distance-loss kernel for Trainium2 (8 NeuronCores, data-parallel over batch).

loss = mean over (b, c != label_b) of sqrt(||x_b - center_c||^2)

Host-side staging is layout/dtype only (fp8/bf16 casts + transposed
tiled views); all arithmetic (norms, matmuls, sqrt, reductions,
correction) runs on device with fp32 accumulation. x-hat = e4m3(x) and
c-hat = e4m3(centers) are used CONSISTENTLY: matmul operands ship as
fp8, every other staged view (row tiles for norms/correction, gather
source) ships as bf16 of the same fp8-rounded values, so
d^2 = ||x||^2 + ||c||^2 - 2 x.c is exact up to the (averaged-out)
quantization of the points themselves.

Per-core plan (B_shard = 2048 rows, distmat computed as psum[c, b]):
  - psum[c, b] = c_c . x_b + (-1/2)||x_b||^2 via fp8 DoubleRow matmuls
    (K=256 per instruction, 2x rate: ~380ns for 512 columns). Centers
    ship RAW and class-padded to 1024: the ScalarE activation computes
    d = sqrt(-2 * psum + ||c_c||^2) with scale=-2 and the class norm as
    per-partition bias, accumulating sum_b d; the pad rows are never
    read.
  - ||x||^2 without transpose or DRAM round-trip: square the resident
    x^T quarter-slices on VectorE as they land, ones-column matmuls
    reduce over d into one psum row [1, 2048], four ScalarE copies with
    scale=-0.5 make the bf16 aug row slices that ride each c-tile's
    psum as K=1 matmuls.
  - label-entry correction: indirect-DMA gather of centers[labels]
    (labels land via the faster-spinning sync queue), sum_d (x-g)^2 on
    VectorE, one sqrt at the end whose output overlaps the m=7 d-tile
    (true WAW) so the scheduler cannot hoist it ahead of the main-loop
    sqrts when the gather chain runs long.
  - A PE warm-up burst runs while DMAs stream so the clock governor
    reaches full speed before the real matmuls.
  - host sums the 8 per-core partials and divides by B*(C-1).
"""

import sys
from contextlib import ExitStack

import numpy as np

if "/opt/trn_rl_repo" not in sys.path:
    sys.path.insert(0, "/opt/trn_rl_repo")

import ml_dtypes

import concourse.bass as bass
import concourse.mybir as mybir
from concourse.bacc import Bacc
from concourse.bass import IndirectOffsetOnAxis
from concourse.tile import TileContext

F32 = mybir.dt.float32
BF16 = mybir.dt.bfloat16
FP8 = mybir.dt.float8e4
I32 = mybir.dt.int32
AF = mybir.ActivationFunctionType
ALU = mybir.AluOpType
PM = mybir.MatmulPerfMode
BF = ml_dtypes.bfloat16
F8 = ml_dtypes.float8_e4m3

N_CORES = 8
B = 16384
C = 1000
CP = 1024                  # classes padded for DoubleRow weight tiles
D = 256
BS = B // N_CORES          # 2048 rows per core
T = BS // 128              # 16 b-tiles per core
NC_TILES = 8               # class tiles
HB = BS // 2               # 1024-column b-half


def build_nc() -> bass.Bass:
    nc = Bacc()
    # xTh: x^T halves (fp8)     xTh[p, i, b] = x[h*1024+b, i*128+p]
    # cT : centers^T (fp8, raw) cT[p, i, c] = centers[c, i*128+p]
    # cpA/cpB: center rows      cpH[p, i*D:(i+1)*D] = centers[(4H+i)*128+p, :]
    # xp0/xp1: x rows, sbuf     xpH[p, t*D:(t+1)*D] = x[(8H+t)*128+p, :]
    # cb : centers rows (indirect-gather source)
    xt_d = {}
    for h, hn in enumerate("ab"):
        for i in range(2):
            xt_d[h, i] = nc.dram_tensor(f"xT{hn}{i}", [128, HB], FP8,
                                        kind="ExternalInput")
    cT_d = nc.dram_tensor("cT", [128, 2, CP], FP8, kind="ExternalInput")
    cpA_d = nc.dram_tensor("cpA", [128, 4 * D], BF16, kind="ExternalInput")
    cpB_d = nc.dram_tensor("cpB", [128, 4 * D], BF16, kind="ExternalInput")
    xp0_d = nc.dram_tensor("xp0", [128, 8 * D], BF16, kind="ExternalInput")
    xp1_d = nc.dram_tensor("xp1", [128, 8 * D], BF16, kind="ExternalInput")
    cb_d = nc.dram_tensor("cb", [C, D], BF16, kind="ExternalInput")
    l_d = nc.dram_tensor("labels", [128, T], I32, kind="ExternalInput")
    o_d = nc.dram_tensor("out", [1, 1], F32, kind="ExternalOutput")

    with TileContext(nc) as tc, ExitStack() as ctx:
        const = ctx.enter_context(tc.tile_pool(name="const", bufs=1))
        cqpool = ctx.enter_context(tc.tile_pool(name="cqpool", bufs=2))
        xpool = ctx.enter_context(tc.tile_pool(name="xpool", bufs=3))
        dpool = ctx.enter_context(tc.tile_pool(name="dpool", bufs=2))
        mmps = ctx.enter_context(tc.tile_pool(name="mmps", bufs=2, space="PSUM"))

        # labels ride the sync queue (it spins up ~1.7us earlier than the
        # scalar queue) so the 16-gather chain on GpSimd starts ASAP.
        lab_sb = const.tile([128, T], I32)
        nc.sync.dma_start(out=lab_sb[:], in_=l_d[:, :])
        g_sb = const.tile([128, T * D], BF16)
        for t in range(T):
            nc.gpsimd.indirect_dma_start(
                out=g_sb[:, t * D : (t + 1) * D],
                out_offset=None,
                in_=cb_d[:, :],
                in_offset=IndirectOffsetOnAxis(ap=lab_sb[:, t : t + 1], axis=0),
            )

        # input DMAs: both k-groups of the a-half early on sync, cT first
        # on scalar, then the b-half; row views + cp follow.
        xTa = const.tile([128, 2, HB], FP8)
        xTb = const.tile([128, 2, HB], FP8)
        nc.sync.dma_start(out=xTa[:, 0, :], in_=xt_d[0, 0][:, :])
        nc.sync.dma_start(out=xTa[:, 1, :], in_=xt_d[0, 1][:, :])
        cTf = const.tile([128, 2, CP], FP8)
        nc.scalar.dma_start(out=cTf[:], in_=cT_d[:, :, :])
        nc.scalar.dma_start(out=xTb[:, 0, :], in_=xt_d[1, 0][:, :])
        nc.scalar.dma_start(out=xTb[:, 1, :], in_=xt_d[1, 1][:, :])
        cpA = const.tile([128, 4 * D], BF16)
        nc.sync.dma_start(out=cpA[:], in_=cpA_d[:, :])
        xp1 = const.tile([128, 8 * D], BF16)
        nc.sync.dma_start(out=xp1[:], in_=xp1_d[:, :])
        cpB = const.tile([128, 4 * D], BF16)
        nc.scalar.dma_start(out=cpB[:], in_=cpB_d[:, :])
        xp0 = const.tile([128, 8 * D], BF16)
        nc.scalar.dma_start(out=xp0[:], in_=xp0_d[:, :])
        xTh = [xTa, xTb]

        def x_slice(t):
            return (xp0 if t < 8 else xp1)[:, (t % 8) * D : (t % 8 + 1) * D]

        # constants on VectorE (queue head, no deps)
        dum0 = const.tile([128, 1], F32)
        nc.vector.memset(dum0[:], 1.0)
        wu_w = const.tile([128, 4], BF16)
        nc.vector.memset(wu_w[:], 0.5)
        wu_r = const.tile([128, 512], BF16)
        nc.vector.memset(wu_r[:], 0.25)
        ones1 = const.tile([1, 128], BF16)
        nc.vector.memset(ones1[:], 1.0)
        onesk = const.tile([128, 1], BF16)
        nc.vector.memset(onesk[:], 1.0)
        acc = const.tile([128, NC_TILES], F32)       # sum_b sqrt(dist)
        nc.vector.memset(acc[:], 0.0)

        # dummy sqrt: loads the Sqrt ACT table before the loop needs it
        dum1 = const.tile([128, 1], F32)
        nc.scalar.activation(dum1[:], dum0[:], AF.Sqrt)

        # PE warm-up burst while DMAs stream; writes junk into the psum
        # row the ones-matmuls later reset via start=True.
        psxx = mmps.tile([128, 2048], F32, tag="mm")
        for rep in range(8):
            nc.tensor.matmul(psxx[0:4, 0:512], wu_w[:], wu_r[:],
                             start=(rep == 0), stop=(rep == 7))

        # ||x||^2 in matmul orientation: square x^T quarter-slices on
        # VectorE as each DMA lands (AP-precise deps), ones-column
        # matmuls reduce over d into psxx[0, j*512:...], one bf16 cast
        # per j with scale=-0.5.
        sqs = {}
        for h, i in ((0, 0), (0, 1), (1, 0), (1, 1)):
            sq_t = cqpool.tile([128, HB], BF16, tag=f"sq{h}{i}")
            nc.vector.tensor_tensor(out=sq_t[:], in0=xTh[h][:, i, :],
                                    in1=xTh[h][:, i, :], op=ALU.mult)
            sqs[h, i] = sq_t
        xx4 = []
        for j in range(4):
            for i in range(2):
                nc.tensor.matmul(
                    psxx[0:1, j * 512 : (j + 1) * 512],
                    onesk[:],
                    sqs[j // 2, i][:, (j % 2) * 512 : (j % 2 + 1) * 512],
                    start=(i == 0), stop=(i == 1),
                )
            xxj = const.tile([1, 512], BF16, tag=f"xx4_{j}")
            nc.scalar.activation(xxj[:], psxx[0:1, j * 512 : (j + 1) * 512],
                                 AF.Copy, scale=-0.5)
            xx4.append(xxj)

        # ||c||^2 per class tile (separate tiles so ACT m only waits on
        # its own column); cpA covers tiles 0-3, cpB tiles 4-7.
        ccs = []
        for i in range(NC_TILES):
            src = (cpA if i < 4 else cpB)[:, (i % 4) * D : (i % 4 + 1) * D]
            cc_i = const.tile([128, 1], F32, tag=f"cc_{i}")
            csq = cqpool.tile([128, D], BF16, tag="csq")
            nc.vector.scalar_tensor_tensor(
                out=csq[:], in0=src, scalar=0.0, in1=src,
                op0=ALU.bypass, op1=ALU.mult,
                accum_out=cc_i[:],
            )
            ccs.append(cc_i)

        # main loop: per c-tile, 4 DoubleRow matmuls (K=256 each) + 4
        # aug matmuls, then ScalarE sqrt(-2*psum + ||c||^2) with row-sum
        # accumulation over the real (non-pad) classes.
        dt_last = None
        for m in range(NC_TILES):
            cnt = min(128, C - m * 128)
            ps = mmps.tile([128, 2048], F32, tag="mm")
            for j in range(4):
                nc.tensor.matmul(
                    ps[0:128, j * 512 : (j + 1) * 512],
                    cTf[:, :, m * 128 : (m + 1) * 128],
                    xTh[j // 2][:, :, (j % 2) * 512 : (j % 2 + 1) * 512],
                    start=True, stop=False,
                    perf_mode=PM.DoubleRow,
                )
            for j in range(4):
                nc.tensor.matmul(
                    ps[0:128, j * 512 : (j + 1) * 512],
                    ones1[:, 0:128],
                    xx4[j][:],
                    start=False, stop=(j == 3),
                )
            dt_ = dpool.tile([128, 2048], BF16, tag="d")
            nc.scalar.activation(
                dt_[0:cnt, :], ps[0:cnt, :], AF.Sqrt,
                bias=ccs[m][0:cnt, :], scale=-2.0,
                accum_out=acc[0:cnt, m : m + 1],
            )
            dt_last = dt_

        # label-entry correction (bf16 operands, fp32 accumulation)
        dacc = const.tile([128, T], F32)             # label-entry dist^2
        for t in range(T):
            df = xpool.tile([128, D], BF16, tag="df")
            nc.vector.tensor_sub(df[:], x_slice(t),
                                 g_sb[:, t * D : (t + 1) * D])
            dfsq = xpool.tile([128, D], BF16, tag="dfsq")
            nc.vector.scalar_tensor_tensor(
                out=dfsq[:], in0=df[:], scalar=0.0, in1=df[:],
                op0=ALU.bypass, op1=ALU.mult, accum_out=dacc[:, t : t + 1],
            )

        # corr sqrt: output overlaps rows the m=7 sqrt wrote (true WAW)
        # so it schedules after the main-loop sqrts; accum_out = sum_t.
        corp = const.tile([128, 1], F32)
        nc.scalar.activation(dt_last[:, 2048 - T :], dacc[:], AF.Sqrt,
                             accum_out=corp[:])
        totp = const.tile([128, 1], F32)
        nc.vector.reduce_sum(out=totp[:], in_=acc[:], axis=mybir.AxisListType.X)
        part = const.tile([128, 1], F32)
        nc.vector.tensor_sub(part[:], totp[:], corp[:])
        ones_col = const.tile([128, 1], F32)
        nc.vector.memset(ones_col[:], 1.0)
        red_ps = mmps.tile([128, 2048], F32, tag="mm")
        nc.tensor.matmul(red_ps[0:1, 0:1], ones_col[:], part[:],
                         start=True, stop=True)
        red = const.tile([1, 1], F32)
        nc.scalar.copy(red[:], red_ps[0:1, 0:1])
        nc.sync.dma_start(out=o_d[0:1, 0:1], in_=red[0:1, 0:1])

    nc.compile()
    return nc


_NC_CACHE = None


def _get_nc():
    global _NC_CACHE
    if _NC_CACHE is None:
        _NC_CACHE = build_nc()
    return _NC_CACHE


def make_in_maps(x, centers, labels):
    x = np.asarray(x, dtype=np.float32)
    centers = np.asarray(centers, dtype=np.float32)
    labels = np.asarray(labels)
    # fp8-rounded points, used consistently by every staged view
    c8pad = np.zeros((CP, D), F8)
    c8pad[:C] = centers.astype(F8)
    cb = c8pad[:C].astype(BF)
    # cT[p, i, c] = c8pad[c, i*128+p]
    cT = np.ascontiguousarray(
        c8pad.T.reshape(2, 128, CP).transpose(1, 0, 2)
    )
    cp = np.ascontiguousarray(
        c8pad.reshape(NC_TILES, 128, D).transpose(1, 0, 2).reshape(128, -1)
    ).astype(BF)
    in_maps = []
    for i in range(N_CORES):
        x8 = x[i * BS : (i + 1) * BS].astype(F8)
        xT = np.ascontiguousarray(x8.T)  # [D, BS] fp8
        xp = np.ascontiguousarray(
            x8.reshape(T, 128, D).transpose(1, 0, 2).reshape(128, -1)
        ).astype(BF)
        ls = labels[i * BS : (i + 1) * BS].astype(np.int32)
        # lab[p, t] = label of shard row t*128 + p (indirect-gather order)
        lab = np.ascontiguousarray(ls.reshape(T, 128).T)
        in_maps.append({
            "xTa0": xT[0:128, 0:HB], "xTa1": xT[128:256, 0:HB],
            "xTb0": xT[0:128, HB:], "xTb1": xT[128:256, HB:],
            "cT": cT, "cpA": cp[:, : 4 * D], "cpB": cp[:, 4 * D :],
            "xp0": xp[:, : 8 * D], "xp1": xp[:, 8 * D :],
            "cb": cb, "labels": lab,
        })
    return in_maps


def _ensure_ntff_hook_module():
    """Provide antenv.axon_hooks if the image's antenv package lacks it.

    concourse.bass_utils imports it for trace=True under axon; the hook
    itself lives in libaxon_pjrt.so and is wrapped by trn_agent_boot.
    """
    import types

    try:
        import antenv.axon_hooks  # noqa: F401
        return
    except ImportError:
        pass
    mod = types.ModuleType("antenv.axon_hooks")
    state = {"hook": None}

    def set_axon_ntff_profile_hook(hook):
        state["hook"] = hook

    def get_axon_ntff_profile_hook():
        if state["hook"] is None:
            try:
                from trn_agent_boot.trn_boot import _ntff_profile_via_ctypes

                state["hook"] = _ntff_profile_via_ctypes(
                    "/opt/axon/libaxon_pjrt.so"
                )
            except Exception:
                return None
        return state["hook"]

    mod.set_axon_ntff_profile_hook = set_axon_ntff_profile_hook
    mod.get_axon_ntff_profile_hook = get_axon_ntff_profile_hook
    sys.modules["antenv.axon_hooks"] = mod
    try:
        import antenv

        antenv.axon_hooks = mod
    except ImportError:
        pass


def kernel(x, centers, labels, _results_out=None, **run_kwargs):
    _ensure_ntff_hook_module()
    from concourse.bass_utils import run_bass_kernel_spmd

    nc = _get_nc()
    in_maps = make_in_maps(x, centers, labels)
    res = run_bass_kernel_spmd(nc, in_maps, core_ids=list(range(N_CORES)),
                               **run_kwargs)
    if _results_out is not None:
        _results_out.append(res)
    partials = [float(r["out"][0, 0]) for r in res.results]
    total = float(np.sum(np.asarray(partials, dtype=np.float64)))
    loss = total / (B * (C - 1))
    return np.float32(loss)
